# revision 12
# baseline (speedup 1.0000x reference)
"""DenseGCNConv on 8 Trainium2 NeuronCores (Bass/Tile).

out = (adj @ features) @ W.T + b,  adj [16384,16384] f32, features [16384,128],
W [128,128], b [128].

Strategy (row-parallel, per the sharding hint): core c owns rows
[c*2048, (c+1)*2048) of adj. Using associativity, out = adj @ fw + b with
fw = features @ W.T computed on-device (replicated on every core - it is
0.5 GFLOP vs 68 GFLOP total). The big operand adj is streamed from HBM
exactly once, as 1-byte fp8 of (adj - 0.5): centering the uniform-[0,1)
entries into [-0.5, 0.5) halves the quantization error. The exact
identity  adj @ fw = (adj - 0.5) @ fw + 0.5 * colsum(fw)  is restored
via a rank-1 correction folded into the bias:
  colsum(fw)[j] = sum_k fw[k,j] = (sum_k features[k,:]) @ W.T[:,j],
computed on device from a free-dim reduction of featT plus one 1-row
matmul.

With a 32 MiB/core fp8 stream the kernel is DMA-bound (~100us at ~370
GB/s/core) only if TensorE stays under that. Measured PE rates (this
hw): plain fp8 matmul = 0.43 ns/row contracting one 128-k chunk; fp8
DoubleRow = 0.43 ns/row contracting TWO chunks (2x work rate), but
DoubleRow needs BOTH operands e4m3 and fw as e4m3-only costs ~1.4e-2
extra error. So the 128 k-chunks are split 50/50, alternating per DMA
group:
  even groups: adj e3m4 x fw bf16, plain matmuls   (accurate, 1x rate)
  odd  groups: adj e4m3 x fw e4m3, DoubleRow pairs (2x rate)
=> PE ~85us < DMA ~100us, end-to-end rel err ~1.35e-2 (gate: 2e-2).
All 128 chunks accumulate into one PSUM region per m-block (mixed-mode
accumulation is just PSUM adds).

TensorE contracts over the partition dimension, so the streamed adj
tiles need K on partitions. The host packs each 1 MiB DMA group so each
partition's slice is contiguous: plain groups as [p, j, m]; DoubleRow
groups as [p, jp, mb, two, m512] so the moving AP [p, 2, 512] reads one
contiguous 1 KiB run per partition. Phase A (fw tiles from featT) is
emitted staggered ~4 groups ahead of first use so the in-order PE queue
never stalls on featT arrival or DVE casts. The per-core output
outT [128 fo, 2048 m] accumulates in 4 PSUM banks; one ACT pass adds
the corrected bias while copying PSUM->SBUF in bf16 (host upcasts).
"""

import sys

if "/opt/trn_rl_repo" not in sys.path:
    sys.path.insert(0, "/opt/trn_rl_repo")

import numpy as np

N = 16384
F = 128
P = 128
CORES = 8
ROWS = N // CORES  # 2048 rows of adj per core
KC = N // P  # 128 k-chunks
CK = 4  # k-chunks per DMA group (1 MiB per dma_start at fp8)
GROUPS = KC // CK  # 32, alternating plain/DoubleRow
MBLK = ROWS // 512  # 4 moving-operand blocks of 512
FEAT_G = N // 2048  # 8 featT DMA groups (also fw tiles, 16 k-chunks each)
ADJ_BUFS = 6  # buffering depth for the adj stream
FW_BUFS = 8  # fw ring depth (8 = fully resident)

_cache = {}


def configure(adj_bufs=None, fw_bufs=None):
    """Experiment knob: change buffering, invalidate caches."""
    global ADJ_BUFS, FW_BUFS
    if adj_bufs is not None:
        ADJ_BUFS = adj_bufs
    if fw_bufs is not None:
        FW_BUFS = fw_bufs
    _cache.clear()


def _split_excess_waits(nc, max_waits=1):
    """Walrus CoreV3 codegen rejects instructions with more than one SyncWait
    ("Too many sync wait commands"). Tile's kernel-tail drain accumulates one
    wait per semaphore lane; hoist the excess onto same-engine NoOps placed
    immediately before the offending instruction."""
    import concourse.mybir as mybir

    counter = [0]

    def fresh_name():
        counter[0] += 1
        return f"I-waitsplit-{counter[0]}"

    for fn in nc.m.functions:
        for blk in fn.blocks:
            new_insts = []
            for inst in blk.instructions:
                si = inst.sync_info
                if si is not None and si.on_wait and len(si.on_wait) > max_waits:
                    waits = list(si.on_wait)
                    extra, keep = waits[:-max_waits], waits[-max_waits:]
                    for i in range(0, len(extra), max_waits):
                        nop = mybir.InstNoOp(
                            name=fresh_name(),
                            engine=inst.engine,
                            sync_info=mybir.SyncInfo(
                                on_wait=extra[i : i + max_waits], on_update=[]
                            ),
                            bass_nofuse=True,
                        )
                        new_insts.append(nop)
                    si.on_wait = keep
                new_insts.append(inst)
            blk.instructions[:] = new_insts


def _build():
    import concourse.bass as bass
    import concourse.mybir as mybir
    from concourse.tile import TileContext

    f32 = mybir.dt.float32
    bf16 = mybir.dt.bfloat16
    e4 = mybir.dt.float8e4  # E4M3
    e3 = mybir.dt.float8e3  # E3M4: 4 mantissa bits
    DR = mybir.MatmulPerfMode.DoubleRow
    nc = bass.Bass()
    # Per-core adj shard, pre-transposed to [k, m] and packed per 1 MiB DMA
    # group; even k-groups (plain path) as e3m4 [g, p, j, m], odd k-groups
    # (DoubleRow path) as e4m3 [g, p, jp, mb, two, m512].
    adjP = nc.declare_dram_parameter(
        "adjP", [(GROUPS // 2) * P, CK * ROWS], e3, isOutput=False
    )
    adjD = nc.declare_dram_parameter(
        "adjD", [(GROUPS // 2) * P, CK * ROWS], e4, isOutput=False
    )
    featT = nc.declare_dram_parameter("featT", [P, N], bf16, isOutput=False)
    wt = nc.declare_dram_parameter("wt", [P, F], bf16, isOutput=False)
    bias = nc.declare_dram_parameter("bias", [P, 1], f32, isOutput=False)
    outT = nc.declare_dram_parameter("outT", [P, ROWS], bf16, isOutput=True)

    with TileContext(nc) as tc:
        with (
            tc.tile_pool(name="const", bufs=1) as const_pool,
            tc.tile_pool(name="feat", bufs=2) as feat_pool,
            tc.tile_pool(name="fw", bufs=FW_BUFS) as fw_pool,
            tc.tile_pool(name="adj", bufs=ADJ_BUFS) as adj_pool,
            tc.tile_pool(name="outp", bufs=1) as out_pool,
            tc.tile_pool(name="psA", bufs=1, space="PSUM") as psA_pool,
            tc.tile_pool(name="psB", bufs=1, space="PSUM") as psB_pool,
        ):
            wt_sb = const_pool.tile([P, F], bf16)
            nc.gpsimd.dma_start(out=wt_sb, in_=wt[:])
            b_sb = const_pool.tile([P, 1], f32)
            nc.gpsimd.dma_start(out=b_sb, in_=bias[:])
            featsum8 = const_pool.tile([P, FEAT_G], f32)

            po = psB_pool.tile([P, ROWS], f32)
            o_sb = out_pool.tile([P, ROWS], bf16)
            adj_rP = adjP[:].rearrange("(G p) f -> G p f", p=P)
            adj_rD = adjD[:].rearrange("(G p) f -> G p f", p=P)

            fw_bf_tiles = []
            fw_hi_tiles = []

            def phase_a_tile(t):
                """fw[k, fo] for k-chunks 16t..16t+15: fp32 PSUM result cast
                to bf16 (plain path) and e4m3 (DoubleRow path). Alongside,
                the featT tile is free-dim-reduced into featsum8[:, t] for
                the centering correction."""
                ft = feat_pool.tile([P, 2048], bf16)
                nc.gpsimd.dma_start(out=ft, in_=featT[:, t * 2048 : (t + 1) * 2048])
                pf = psA_pool.tile([P, 2048], f32, tag="pf")
                for j in range(2048 // F):
                    nc.tensor.matmul(
                        pf[:, j * F : (j + 1) * F],
                        lhsT=ft[:, j * F : (j + 1) * F],
                        rhs=wt_sb,
                        start=True,
                        stop=True,
                    )
                fw_bf = fw_pool.tile([P, 2048], bf16, tag="fwbf")
                nc.vector.tensor_copy(out=fw_bf, in_=pf)
                fw_hi = fw_pool.tile([P, 2048], e4, tag="fwhi")
                nc.vector.tensor_copy(out=fw_hi, in_=pf)
                nc.vector.tensor_reduce(
                    out=featsum8[:, t : t + 1],
                    in_=ft,
                    axis=mybir.AxisListType.X,
                    op=mybir.AluOpType.add,
                )
                fw_bf_tiles.append(fw_bf.rearrange("p (c f) -> p c f", f=F))
                fw_hi_tiles.append(fw_hi.rearrange("p (c f) -> p c f", f=F))

            def mm_plain(at, g, j, mb):
                ck = g * CK + j
                w = fw_bf_tiles[ck // 16]
                nc.tensor.matmul(
                    po[:, mb * 512 : (mb + 1) * 512],
                    lhsT=w[:, ck % 16, :],
                    rhs=at[:, j * ROWS + mb * 512 : j * ROWS + (mb + 1) * 512],
                    start=(ck == 0),
                    stop=False,
                )

            def mm_dr(at5, g, jp, mb):
                """DoubleRow: contracts k-chunks (ck, ck+1) for m-block mb."""
                ck = g * CK + 2 * jp
                w = fw_hi_tiles[ck // 16]
                c = ck % 16
                nc.tensor.matmul(
                    po[:, mb * 512 : (mb + 1) * 512],
                    lhsT=w[:, c : c + 2, :],
                    rhs=at5[:, jp, mb, :, :],
                    start=False,
                    stop=(ck == KC - 2),
                    perf_mode=DR,
                )

            phase_a_tile(0)
            phase_a_tile(1)
            for g in range(GROUPS):
                # Keep fw production one tile ahead of consumption.
                gpt = 16 // CK
                t = g // gpt
                if g % gpt == 0 and t >= 1 and t + 1 < FEAT_G:
                    phase_a_tile(t + 1)
                plain = g % 2 == 0
                at = adj_pool.tile([P, CK * ROWS], e3 if plain else e4)
                dma_eng = nc.sync if plain else nc.scalar
                dma_eng.dma_start(
                    out=at, in_=(adj_rP if plain else adj_rD)[g // 2]
                )
                if g == GROUPS - 1:
                    # Centering correction: bias_eff = b + 0.5*(featsum@W.T).
                    featsum = const_pool.tile([P, 1], f32)
                    nc.vector.tensor_reduce(
                        out=featsum,
                        in_=featsum8,
                        axis=mybir.AxisListType.X,
                        op=mybir.AluOpType.add,
                    )
                    featsum_bf = const_pool.tile([P, 1], bf16)
                    nc.vector.tensor_copy(out=featsum_bf, in_=featsum)
                    # Same tag as pf so the bufs=1 ring reuses the slot
                    # instead of growing past the 8 PSUM banks; col 0 used.
                    psC_big = psA_pool.tile([P, 2048], f32, tag="pf")
                    psC = psC_big[:, 0:1]
                    nc.tensor.matmul(
                        psC, lhsT=wt_sb, rhs=featsum_bf, start=True, stop=True
                    )
                    bias_eff = const_pool.tile([P, 1], f32)
                    nc.scalar.activation(
                        bias_eff,
                        psC,
                        mybir.ActivationFunctionType.Identity,
                        bias=b_sb,
                        scale=0.5,
                    )
                    # Last group (odd => DoubleRow): finish one m-block at a
                    # time so the bias-add and output DMA of block mb overlap
                    # the matmuls of mb+1.
                    at5 = at.rearrange(
                        "p (jp mb two m) -> p jp mb two m", jp=CK // 2, mb=MBLK, two=2
                    )
                    for mb in range(MBLK):
                        for jp in range(CK // 2):
                            mm_dr(at5, g, jp, mb)
                        sl = slice(mb * 512, (mb + 1) * 512)
                        nc.scalar.activation(
                            o_sb[:, sl],
                            po[:, sl],
                            mybir.ActivationFunctionType.Identity,
                            bias=bias_eff,
                            scale=1.0,
                        )
                        nc.gpsimd.dma_start(out=outT[:, sl], in_=o_sb[:, sl])
                elif plain:
                    for j in range(CK):
                        for mb in range(MBLK):
                            mm_plain(at, g, j, mb)
                else:
                    at5 = at.rearrange(
                        "p (jp mb two m) -> p jp mb two m", jp=CK // 2, mb=MBLK, two=2
                    )
                    for jp in range(CK // 2):
                        for mb in range(MBLK):
                            mm_dr(at5, g, jp, mb)

    _split_excess_waits(nc)
    return nc


def _get_nc():
    if "nc" not in _cache:
        _cache["nc"] = _build()
    return _cache["nc"]


def make_in_maps(adj, features, W, b):
    import ml_dtypes

    adj = np.asarray(adj, dtype=np.float32)
    features = np.asarray(features, dtype=np.float32)
    W = np.asarray(W, dtype=np.float32)
    b = np.asarray(b, dtype=np.float32)

    featT = np.ascontiguousarray(features.T.astype(ml_dtypes.bfloat16))  # [fi, k]
    wt = np.ascontiguousarray(W.T.astype(ml_dtypes.bfloat16))  # [fi, fo]
    bias = np.ascontiguousarray(b.reshape(P, 1))

    in_maps = []
    half = GROUPS // 2
    for c in range(CORES):
        # [k, m] transpose of the row shard, centered; split into the even
        # (plain/e3m4) and odd (DoubleRow/e4m3) k-groups.
        arrT = np.ascontiguousarray(
            adj[c * ROWS : (c + 1) * ROWS, :].T
        ) - np.float32(0.5)
        bands = arrT.reshape(GROUPS, CK * P, ROWS)
        # plain: [gi, (j p), m] -> [gi, p, j, m]
        shardP = (
            bands[0::2]
            .astype(ml_dtypes.float8_e3m4)
            .reshape(half, CK, P, ROWS)
            .transpose(0, 2, 1, 3)
            .reshape(half * P, CK * ROWS)
        )
        # DoubleRow: [gi, (jp two p), m] -> [gi, p, jp, mb, two, m512]
        shardD = (
            bands[1::2]
            .astype(ml_dtypes.float8_e4m3)
            .reshape(half, CK // 2, 2, P, MBLK, 512)
            .transpose(0, 3, 1, 4, 2, 5)
            .reshape(half * P, CK * ROWS)
        )
        in_maps.append(
            {"adjP": shardP, "adjD": shardD, "featT": featT, "wt": wt, "bias": bias}
        )
    return in_maps


def assemble_output(results):
    out = np.empty((N, F), dtype=np.float32)
    for c in range(CORES):
        out[c * ROWS : (c + 1) * ROWS, :] = results[c]["outT"].astype(np.float32).T
    return out


def kernel(adj, features, W, b):
    from concourse.bass_utils import run_bass_kernel_spmd

    nc = _get_nc()
    in_maps = make_in_maps(adj, features, W, b)
    res = run_bass_kernel_spmd(nc, in_maps, list(range(CORES)))
    return assemble_output(res.results)


# revision 13
# speedup vs baseline: 1.0264x; 1.0264x over previous
"""DenseGCNConv on 8 Trainium2 NeuronCores (Bass/Tile).

out = (adj @ features) @ W.T + b,  adj [16384,16384] f32, features [16384,128],
W [128,128], b [128].

Strategy (row-parallel, per the sharding hint): core c owns rows
[c*2048, (c+1)*2048) of adj. Using associativity, out = adj @ fw + b with
fw = features @ W.T computed on-device (replicated on every core - it is
0.5 GFLOP vs 68 GFLOP total). The big operand adj is streamed from HBM
exactly once, as 1-byte fp8 of (adj - 0.5): centering the uniform-[0,1)
entries into [-0.5, 0.5) halves the quantization error. The exact
identity  adj @ fw = (adj - 0.5) @ fw + 0.5 * colsum(fw)  is restored
via a rank-1 correction folded into the bias:
  colsum(fw)[j] = sum_k fw[k,j] = (sum_k features[k,:]) @ W.T[:,j],
computed on device from a free-dim reduction of featT plus one 1-row
matmul.

With a 32 MiB/core fp8 stream the kernel is DMA-bound (~100us at ~370
GB/s/core) only if TensorE stays under that. Measured PE rates (this
hw): plain fp8 matmul = 0.43 ns/row contracting one 128-k chunk; fp8
DoubleRow = 0.43 ns/row contracting TWO chunks (2x work rate), but
DoubleRow needs BOTH operands e4m3 and fw as e4m3-only costs ~1.4e-2
extra error. So the 128 k-chunks are split 50/50, alternating per DMA
group:
  even groups: adj e3m4 x fw bf16, plain matmuls   (accurate, 1x rate)
  odd  groups: adj e4m3 x fw e4m3, DoubleRow pairs (2x rate)
=> PE ~85us < DMA ~100us, end-to-end rel err ~1.35e-2 (gate: 2e-2).
All 128 chunks accumulate into one PSUM region per m-block (mixed-mode
accumulation is just PSUM adds).

TensorE contracts over the partition dimension, so the streamed adj
tiles need K on partitions. The host packs each 1 MiB DMA group so each
partition's slice is contiguous: plain groups as [p, j, m]; DoubleRow
groups as [p, jp, mb, two, m512] so the moving AP [p, 2, 512] reads one
contiguous 1 KiB run per partition. Phase A (fw tiles from featT) is
emitted staggered ~4 groups ahead of first use so the in-order PE queue
never stalls on featT arrival or DVE casts. The per-core output
outT [128 fo, 2048 m] accumulates in 4 PSUM banks; one ACT pass adds
the corrected bias while copying PSUM->SBUF in bf16 (host upcasts).
"""

import sys

if "/opt/trn_rl_repo" not in sys.path:
    sys.path.insert(0, "/opt/trn_rl_repo")

import numpy as np

N = 16384
F = 128
P = 128
CORES = 8
ROWS = N // CORES  # 2048 rows of adj per core
KC = N // P  # 128 k-chunks
CK = 16  # k-chunks per DMA group (4 MiB per dma_start at fp8)
GROUPS = KC // CK  # 32, alternating plain/DoubleRow
MBLK = ROWS // 512  # 4 moving-operand blocks of 512
FEAT_G = N // 2048  # 8 featT DMA groups (also fw tiles, 16 k-chunks each)
ADJ_BUFS = 3  # buffering depth for the adj stream (in CK-chunk groups)
FW_BUFS = 8  # fw ring depth (8 = fully resident)

_cache = {}


def configure(adj_bufs=None, fw_bufs=None):
    """Experiment knob: change buffering, invalidate caches."""
    global ADJ_BUFS, FW_BUFS
    if adj_bufs is not None:
        ADJ_BUFS = adj_bufs
    if fw_bufs is not None:
        FW_BUFS = fw_bufs
    _cache.clear()


def _split_excess_waits(nc, max_waits=1):
    """Walrus CoreV3 codegen rejects instructions with more than one SyncWait
    ("Too many sync wait commands"). Tile's kernel-tail drain accumulates one
    wait per semaphore lane; hoist the excess onto same-engine NoOps placed
    immediately before the offending instruction."""
    import concourse.mybir as mybir

    counter = [0]

    def fresh_name():
        counter[0] += 1
        return f"I-waitsplit-{counter[0]}"

    for fn in nc.m.functions:
        for blk in fn.blocks:
            new_insts = []
            for inst in blk.instructions:
                si = inst.sync_info
                if si is not None and si.on_wait and len(si.on_wait) > max_waits:
                    waits = list(si.on_wait)
                    extra, keep = waits[:-max_waits], waits[-max_waits:]
                    for i in range(0, len(extra), max_waits):
                        nop = mybir.InstNoOp(
                            name=fresh_name(),
                            engine=inst.engine,
                            sync_info=mybir.SyncInfo(
                                on_wait=extra[i : i + max_waits], on_update=[]
                            ),
                            bass_nofuse=True,
                        )
                        new_insts.append(nop)
                    si.on_wait = keep
                new_insts.append(inst)
            blk.instructions[:] = new_insts


def _build():
    import concourse.bass as bass
    import concourse.mybir as mybir
    from concourse.tile import TileContext

    f32 = mybir.dt.float32
    bf16 = mybir.dt.bfloat16
    e4 = mybir.dt.float8e4  # E4M3
    e3 = mybir.dt.float8e3  # E3M4: 4 mantissa bits
    DR = mybir.MatmulPerfMode.DoubleRow
    nc = bass.Bass()
    # Per-core adj shard, pre-transposed to [k, m] and packed per 1 MiB DMA
    # group; even k-groups (plain path) as e3m4 [g, p, j, m], odd k-groups
    # (DoubleRow path) as e4m3 [g, p, jp, mb, two, m512].
    adjP = nc.declare_dram_parameter(
        "adjP", [(GROUPS // 2) * P, CK * ROWS], e3, isOutput=False
    )
    adjD = nc.declare_dram_parameter(
        "adjD", [(GROUPS // 2) * P, CK * ROWS], e4, isOutput=False
    )
    featT = nc.declare_dram_parameter("featT", [P, N], bf16, isOutput=False)
    wt = nc.declare_dram_parameter("wt", [P, F], bf16, isOutput=False)
    bias = nc.declare_dram_parameter("bias", [P, 1], f32, isOutput=False)
    outT = nc.declare_dram_parameter("outT", [P, ROWS], bf16, isOutput=True)

    with TileContext(nc) as tc:
        with (
            tc.tile_pool(name="const", bufs=1) as const_pool,
            tc.tile_pool(name="feat", bufs=2) as feat_pool,
            tc.tile_pool(name="fw", bufs=FW_BUFS) as fw_pool,
            tc.tile_pool(name="adj", bufs=ADJ_BUFS) as adj_pool,
            tc.tile_pool(name="outp", bufs=1) as out_pool,
            tc.tile_pool(name="psA", bufs=1, space="PSUM") as psA_pool,
            tc.tile_pool(name="psB", bufs=1, space="PSUM") as psB_pool,
        ):
            wt_sb = const_pool.tile([P, F], bf16)
            nc.scalar.dma_start(out=wt_sb, in_=wt[:])
            b_sb = const_pool.tile([P, 1], f32)
            nc.scalar.dma_start(out=b_sb, in_=bias[:])
            featsum8 = const_pool.tile([P, FEAT_G], f32)

            po = psB_pool.tile([P, ROWS], f32)
            o_sb = out_pool.tile([P, ROWS], bf16)
            adj_rP = adjP[:].rearrange("(G p) f -> G p f", p=P)
            adj_rD = adjD[:].rearrange("(G p) f -> G p f", p=P)

            fw_bf_tiles = []
            fw_hi_tiles = []

            def phase_a_tile(t):
                """fw[k, fo] for k-chunks 16t..16t+15: fp32 PSUM result cast
                to bf16 (plain path) and e4m3 (DoubleRow path). Alongside,
                the featT tile is free-dim-reduced into featsum8[:, t] for
                the centering correction."""
                ft = feat_pool.tile([P, 2048], bf16)
                eng = nc.sync if t % 2 == 0 else nc.scalar
                eng.dma_start(out=ft, in_=featT[:, t * 2048 : (t + 1) * 2048])
                pf = psA_pool.tile([P, 2048], f32, tag="pf")
                for j in range(2048 // F):
                    nc.tensor.matmul(
                        pf[:, j * F : (j + 1) * F],
                        lhsT=ft[:, j * F : (j + 1) * F],
                        rhs=wt_sb,
                        start=True,
                        stop=True,
                    )
                fw_bf = fw_pool.tile([P, 2048], bf16, tag="fwbf")
                nc.vector.tensor_copy(out=fw_bf, in_=pf)
                fw_hi = fw_pool.tile([P, 2048], e4, tag="fwhi")
                nc.vector.tensor_copy(out=fw_hi, in_=pf)
                nc.vector.tensor_reduce(
                    out=featsum8[:, t : t + 1],
                    in_=ft,
                    axis=mybir.AxisListType.X,
                    op=mybir.AluOpType.add,
                )
                fw_bf_tiles.append(fw_bf.rearrange("p (c f) -> p c f", f=F))
                fw_hi_tiles.append(fw_hi.rearrange("p (c f) -> p c f", f=F))

            def mm_plain(at, g, j, mb):
                ck = g * CK + j
                w = fw_bf_tiles[ck // 16]
                nc.tensor.matmul(
                    po[:, mb * 512 : (mb + 1) * 512],
                    lhsT=w[:, ck % 16, :],
                    rhs=at[:, j * ROWS + mb * 512 : j * ROWS + (mb + 1) * 512],
                    start=(ck == 0),
                    stop=False,
                )

            def mm_dr(at5, g, jp, mb):
                """DoubleRow: contracts k-chunks (ck, ck+1) for m-block mb."""
                ck = g * CK + 2 * jp
                w = fw_hi_tiles[ck // 16]
                c = ck % 16
                nc.tensor.matmul(
                    po[:, mb * 512 : (mb + 1) * 512],
                    lhsT=w[:, c : c + 2, :],
                    rhs=at5[:, jp, mb, :, :],
                    start=False,
                    stop=(ck == KC - 2),
                    perf_mode=DR,
                )

            phase_a_tile(0)
            phase_a_tile(1)
            for g in range(GROUPS):
                # Keep fw production one tile ahead of consumption.
                gpt = 16 // CK
                t = g // gpt
                if g % gpt == 0 and t >= 1 and t + 1 < FEAT_G:
                    phase_a_tile(t + 1)
                plain = g % 2 == 0
                at = adj_pool.tile([P, CK * ROWS], e3 if plain else e4)
                dma_eng = nc.sync if plain else nc.scalar
                dma_eng.dma_start(
                    out=at, in_=(adj_rP if plain else adj_rD)[g // 2]
                )
                if g == GROUPS - 1:
                    # Centering correction: bias_eff = b + 0.5*(featsum@W.T).
                    featsum = const_pool.tile([P, 1], f32)
                    nc.vector.tensor_reduce(
                        out=featsum,
                        in_=featsum8,
                        axis=mybir.AxisListType.X,
                        op=mybir.AluOpType.add,
                    )
                    featsum_bf = const_pool.tile([P, 1], bf16)
                    nc.vector.tensor_copy(out=featsum_bf, in_=featsum)
                    # Same tag as pf so the bufs=1 ring reuses the slot
                    # instead of growing past the 8 PSUM banks; col 0 used.
                    psC_big = psA_pool.tile([P, 2048], f32, tag="pf")
                    psC = psC_big[:, 0:1]
                    nc.tensor.matmul(
                        psC, lhsT=wt_sb, rhs=featsum_bf, start=True, stop=True
                    )
                    bias_eff = const_pool.tile([P, 1], f32)
                    nc.scalar.activation(
                        bias_eff,
                        psC,
                        mybir.ActivationFunctionType.Identity,
                        bias=b_sb,
                        scale=0.5,
                    )
                    # Last group (odd => DoubleRow): finish one m-block at a
                    # time so the bias-add and output DMA of block mb overlap
                    # the matmuls of mb+1.
                    at5 = at.rearrange(
                        "p (jp mb two m) -> p jp mb two m", jp=CK // 2, mb=MBLK, two=2
                    )
                    for mb in range(MBLK):
                        for jp in range(CK // 2):
                            mm_dr(at5, g, jp, mb)
                        sl = slice(mb * 512, (mb + 1) * 512)
                        nc.scalar.activation(
                            o_sb[:, sl],
                            po[:, sl],
                            mybir.ActivationFunctionType.Identity,
                            bias=bias_eff,
                            scale=1.0,
                        )
                        nc.sync.dma_start(out=outT[:, sl], in_=o_sb[:, sl])
                elif plain:
                    for j in range(CK):
                        for mb in range(MBLK):
                            mm_plain(at, g, j, mb)
                else:
                    at5 = at.rearrange(
                        "p (jp mb two m) -> p jp mb two m", jp=CK // 2, mb=MBLK, two=2
                    )
                    for jp in range(CK // 2):
                        for mb in range(MBLK):
                            mm_dr(at5, g, jp, mb)

    _split_excess_waits(nc)
    return nc


def _get_nc():
    if "nc" not in _cache:
        _cache["nc"] = _build()
    return _cache["nc"]


def make_in_maps(adj, features, W, b):
    import ml_dtypes

    adj = np.asarray(adj, dtype=np.float32)
    features = np.asarray(features, dtype=np.float32)
    W = np.asarray(W, dtype=np.float32)
    b = np.asarray(b, dtype=np.float32)

    featT = np.ascontiguousarray(features.T.astype(ml_dtypes.bfloat16))  # [fi, k]
    wt = np.ascontiguousarray(W.T.astype(ml_dtypes.bfloat16))  # [fi, fo]
    bias = np.ascontiguousarray(b.reshape(P, 1))

    in_maps = []
    half = GROUPS // 2
    for c in range(CORES):
        # [k, m] transpose of the row shard, centered; split into the even
        # (plain/e3m4) and odd (DoubleRow/e4m3) k-groups.
        arrT = np.ascontiguousarray(
            adj[c * ROWS : (c + 1) * ROWS, :].T
        ) - np.float32(0.5)
        bands = arrT.reshape(GROUPS, CK * P, ROWS)
        # plain: [gi, (j p), m] -> [gi, p, j, m]
        shardP = (
            bands[0::2]
            .astype(ml_dtypes.float8_e3m4)
            .reshape(half, CK, P, ROWS)
            .transpose(0, 2, 1, 3)
            .reshape(half * P, CK * ROWS)
        )
        # DoubleRow: [gi, (jp two p), m] -> [gi, p, jp, mb, two, m512]
        shardD = (
            bands[1::2]
            .astype(ml_dtypes.float8_e4m3)
            .reshape(half, CK // 2, 2, P, MBLK, 512)
            .transpose(0, 3, 1, 4, 2, 5)
            .reshape(half * P, CK * ROWS)
        )
        in_maps.append(
            {"adjP": shardP, "adjD": shardD, "featT": featT, "wt": wt, "bias": bias}
        )
    return in_maps


def assemble_output(results):
    out = np.empty((N, F), dtype=np.float32)
    for c in range(CORES):
        out[c * ROWS : (c + 1) * ROWS, :] = results[c]["outT"].astype(np.float32).T
    return out


def kernel(adj, features, W, b):
    from concourse.bass_utils import run_bass_kernel_spmd

    nc = _get_nc()
    in_maps = make_in_maps(adj, features, W, b)
    res = run_bass_kernel_spmd(nc, in_maps, list(range(CORES)))
    return assemble_output(res.results)


# revision 14
# speedup vs baseline: 1.1190x; 1.0901x over previous
"""DenseGCNConv on 8 Trainium2 NeuronCores (Bass/Tile).

out = (adj @ features) @ W.T + b,  adj [16384,16384] f32, features [16384,128],
W [128,128], b [128].

Strategy (row-parallel, per the sharding hint): core c owns rows
[c*2048, (c+1)*2048) of adj. Using associativity, out = adj @ fw + b with
fw = features @ W.T computed on-device (replicated on every core - it is
0.5 GFLOP vs 68 GFLOP total). The big operand adj is streamed from HBM
exactly once, as 1-byte fp8 of (adj - 0.5): centering the uniform-[0,1)
entries into [-0.5, 0.5) halves the quantization error. The exact
identity  adj @ fw = (adj - 0.5) @ fw + 0.5 * colsum(fw)  is restored
via a rank-1 correction folded into the bias:
  colsum(fw)[j] = sum_k fw[k,j] = (sum_k features[k,:]) @ W.T[:,j],
computed on device from a free-dim reduction of featT plus one 1-row
matmul.

With a 32 MiB/core fp8 stream the kernel is DMA-bound (~100us at ~370
GB/s/core) only if TensorE stays under that. Measured PE rates (this
hw): plain fp8 matmul = 0.43 ns/row contracting one 128-k chunk; fp8
DoubleRow = 0.43 ns/row contracting TWO chunks (2x work rate), but
DoubleRow needs BOTH operands e4m3 and fw as e4m3-only costs ~1.4e-2
extra error. So the 128 k-chunks are split 50/50, alternating per DMA
group:
  even groups: adj e3m4 x fw bf16, plain matmuls   (accurate, 1x rate)
  odd  groups: adj e4m3 x fw e4m3, DoubleRow pairs (2x rate)
=> PE ~85us < DMA ~100us, end-to-end rel err ~1.35e-2 (gate: 2e-2).
All 128 chunks accumulate into one PSUM region per m-block (mixed-mode
accumulation is just PSUM adds).

TensorE contracts over the partition dimension, so the streamed adj
tiles need K on partitions. The host packs each 1 MiB DMA group so each
partition's slice is contiguous: plain groups as [p, j, m]; DoubleRow
groups as [p, jp, mb, two, m512] so the moving AP [p, 2, 512] reads one
contiguous 1 KiB run per partition. Phase A (fw tiles from featT) is
emitted staggered ~4 groups ahead of first use so the in-order PE queue
never stalls on featT arrival or DVE casts. The per-core output
outT [128 fo, 2048 m] accumulates in 4 PSUM banks; one ACT pass adds
the corrected bias while copying PSUM->SBUF in bf16 (host upcasts).
"""

import sys

if "/opt/trn_rl_repo" not in sys.path:
    sys.path.insert(0, "/opt/trn_rl_repo")

import numpy as np

N = 16384
F = 128
P = 128
CORES = 8
ROWS = N // CORES  # 2048 rows of adj per core
KC = N // P  # 128 k-chunks
CK = 4  # k-chunks per DMA group (1 MiB per dma_start at fp8)
GROUPS = KC // CK  # 32, alternating plain/DoubleRow
MBLK = ROWS // 512  # 4 moving-operand blocks of 512
FEAT_G = N // 2048  # 8 featT DMA groups (also fw tiles, 16 k-chunks each)
ADJ_BUFS = 6  # buffering depth for the adj stream
FW_BUFS = 8  # fw ring depth (8 = fully resident)

_cache = {}


def configure(adj_bufs=None, fw_bufs=None):
    """Experiment knob: change buffering, invalidate caches."""
    global ADJ_BUFS, FW_BUFS
    if adj_bufs is not None:
        ADJ_BUFS = adj_bufs
    if fw_bufs is not None:
        FW_BUFS = fw_bufs
    _cache.clear()


def _split_excess_waits(nc, max_waits=1):
    """Walrus CoreV3 codegen rejects instructions with more than one SyncWait
    ("Too many sync wait commands"). Tile's kernel-tail drain accumulates one
    wait per semaphore lane; hoist the excess onto same-engine NoOps placed
    immediately before the offending instruction."""
    import concourse.mybir as mybir

    counter = [0]

    def fresh_name():
        counter[0] += 1
        return f"I-waitsplit-{counter[0]}"

    for fn in nc.m.functions:
        for blk in fn.blocks:
            new_insts = []
            for inst in blk.instructions:
                si = inst.sync_info
                if si is not None and si.on_wait and len(si.on_wait) > max_waits:
                    waits = list(si.on_wait)
                    extra, keep = waits[:-max_waits], waits[-max_waits:]
                    for i in range(0, len(extra), max_waits):
                        nop = mybir.InstNoOp(
                            name=fresh_name(),
                            engine=inst.engine,
                            sync_info=mybir.SyncInfo(
                                on_wait=extra[i : i + max_waits], on_update=[]
                            ),
                            bass_nofuse=True,
                        )
                        new_insts.append(nop)
                    si.on_wait = keep
                new_insts.append(inst)
            blk.instructions[:] = new_insts


def _build():
    import concourse.bass as bass
    import concourse.mybir as mybir
    from concourse.tile import TileContext

    f32 = mybir.dt.float32
    bf16 = mybir.dt.bfloat16
    e4 = mybir.dt.float8e4  # E4M3
    e3 = mybir.dt.float8e3  # E3M4: 4 mantissa bits
    DR = mybir.MatmulPerfMode.DoubleRow
    nc = bass.Bass()
    # Per-core adj shard, pre-transposed to [k, m] and packed per 1 MiB DMA
    # group; even k-groups (plain path) as e3m4 [g, p, j, m], odd k-groups
    # (DoubleRow path) as e4m3 [g, p, jp, mb, two, m512].
    adjP = nc.declare_dram_parameter(
        "adjP", [(GROUPS // 2) * P, CK * ROWS], e3, isOutput=False
    )
    adjD = nc.declare_dram_parameter(
        "adjD", [(GROUPS // 2) * P, CK * ROWS], e4, isOutput=False
    )
    featT = nc.declare_dram_parameter("featT", [P, N], bf16, isOutput=False)
    wt = nc.declare_dram_parameter("wt", [P, F], bf16, isOutput=False)
    bias = nc.declare_dram_parameter("bias", [P, 1], f32, isOutput=False)
    outT = nc.declare_dram_parameter("outT", [P, ROWS], bf16, isOutput=True)

    with TileContext(nc) as tc:
        with (
            tc.tile_pool(name="const", bufs=1) as const_pool,
            tc.tile_pool(name="feat", bufs=2) as feat_pool,
            tc.tile_pool(name="fw", bufs=FW_BUFS) as fw_pool,
            tc.tile_pool(name="adj", bufs=ADJ_BUFS) as adj_pool,
            tc.tile_pool(name="outp", bufs=1) as out_pool,
            tc.tile_pool(name="psA", bufs=1, space="PSUM") as psA_pool,
            tc.tile_pool(name="psB", bufs=1, space="PSUM") as psB_pool,
        ):
            wt_sb = const_pool.tile([P, F], bf16)
            nc.scalar.dma_start(out=wt_sb, in_=wt[:])
            b_sb = const_pool.tile([P, 1], f32)
            nc.scalar.dma_start(out=b_sb, in_=bias[:])
            featsum8 = const_pool.tile([P, FEAT_G], f32)

            po = psB_pool.tile([P, ROWS], f32)
            o_sb = out_pool.tile([P, ROWS], bf16)
            adj_rP = adjP[:].rearrange("(G p) f -> G p f", p=P)
            adj_rD = adjD[:].rearrange("(G p) f -> G p f", p=P)

            fw_bf_tiles = []
            fw_hi_tiles = []

            def phase_a_tile(t):
                """fw[k, fo] for k-chunks 16t..16t+15: fp32 PSUM result cast
                to bf16 (plain path) and e4m3 (DoubleRow path). Alongside,
                the featT tile is free-dim-reduced into featsum8[:, t] for
                the centering correction."""
                ft = feat_pool.tile([P, 2048], bf16)
                eng = nc.sync if t % 2 == 0 else nc.scalar
                eng.dma_start(out=ft, in_=featT[:, t * 2048 : (t + 1) * 2048])
                pf = psA_pool.tile([P, 2048], f32, tag="pf")
                for j in range(2048 // F):
                    nc.tensor.matmul(
                        pf[:, j * F : (j + 1) * F],
                        lhsT=ft[:, j * F : (j + 1) * F],
                        rhs=wt_sb,
                        start=True,
                        stop=True,
                    )
                fw_bf = fw_pool.tile([P, 2048], bf16, tag="fwbf")
                nc.vector.tensor_copy(out=fw_bf, in_=pf)
                fw_hi = fw_pool.tile([P, 2048], e4, tag="fwhi")
                nc.vector.tensor_copy(out=fw_hi, in_=pf)
                nc.vector.tensor_reduce(
                    out=featsum8[:, t : t + 1],
                    in_=ft,
                    axis=mybir.AxisListType.X,
                    op=mybir.AluOpType.add,
                )
                fw_bf_tiles.append(fw_bf.rearrange("p (c f) -> p c f", f=F))
                fw_hi_tiles.append(fw_hi.rearrange("p (c f) -> p c f", f=F))

            def mm_plain(at, g, j, mb):
                ck = g * CK + j
                w = fw_bf_tiles[ck // 16]
                nc.tensor.matmul(
                    po[:, mb * 512 : (mb + 1) * 512],
                    lhsT=w[:, ck % 16, :],
                    rhs=at[:, j * ROWS + mb * 512 : j * ROWS + (mb + 1) * 512],
                    start=(ck == 0),
                    stop=False,
                )

            def mm_dr(at5, g, jp, mb):
                """DoubleRow: contracts k-chunks (ck, ck+1) for m-block mb."""
                ck = g * CK + 2 * jp
                w = fw_hi_tiles[ck // 16]
                c = ck % 16
                nc.tensor.matmul(
                    po[:, mb * 512 : (mb + 1) * 512],
                    lhsT=w[:, c : c + 2, :],
                    rhs=at5[:, jp, mb, :, :],
                    start=False,
                    stop=(ck == KC - 2),
                    perf_mode=DR,
                )

            phase_a_tile(0)
            phase_a_tile(1)
            for g in range(GROUPS):
                # Keep fw production one tile ahead of consumption.
                gpt = 16 // CK
                t = g // gpt
                if g % gpt == 0 and t >= 1 and t + 1 < FEAT_G:
                    phase_a_tile(t + 1)
                plain = g % 2 == 0
                at = adj_pool.tile([P, CK * ROWS], e3 if plain else e4)
                dma_eng = nc.sync if plain else nc.scalar
                dma_eng.dma_start(
                    out=at, in_=(adj_rP if plain else adj_rD)[g // 2]
                )
                if g == GROUPS - 1:
                    # Centering correction: bias_eff = b + 0.5*(featsum@W.T).
                    featsum = const_pool.tile([P, 1], f32)
                    nc.vector.tensor_reduce(
                        out=featsum,
                        in_=featsum8,
                        axis=mybir.AxisListType.X,
                        op=mybir.AluOpType.add,
                    )
                    featsum_bf = const_pool.tile([P, 1], bf16)
                    nc.vector.tensor_copy(out=featsum_bf, in_=featsum)
                    # Same tag as pf so the bufs=1 ring reuses the slot
                    # instead of growing past the 8 PSUM banks; col 0 used.
                    psC_big = psA_pool.tile([P, 2048], f32, tag="pf")
                    psC = psC_big[:, 0:1]
                    nc.tensor.matmul(
                        psC, lhsT=wt_sb, rhs=featsum_bf, start=True, stop=True
                    )
                    bias_eff = const_pool.tile([P, 1], f32)
                    nc.scalar.activation(
                        bias_eff,
                        psC,
                        mybir.ActivationFunctionType.Identity,
                        bias=b_sb,
                        scale=0.5,
                    )
                    # Last group (odd => DoubleRow): finish one m-block at a
                    # time so the bias-add and output DMA of block mb overlap
                    # the matmuls of mb+1.
                    at5 = at.rearrange(
                        "p (jp mb two m) -> p jp mb two m", jp=CK // 2, mb=MBLK, two=2
                    )
                    for mb in range(MBLK):
                        for jp in range(CK // 2):
                            mm_dr(at5, g, jp, mb)
                        sl = slice(mb * 512, (mb + 1) * 512)
                        nc.scalar.activation(
                            o_sb[:, sl],
                            po[:, sl],
                            mybir.ActivationFunctionType.Identity,
                            bias=bias_eff,
                            scale=1.0,
                        )
                        nc.sync.dma_start(out=outT[:, sl], in_=o_sb[:, sl])
                elif plain:
                    for j in range(CK):
                        for mb in range(MBLK):
                            mm_plain(at, g, j, mb)
                else:
                    at5 = at.rearrange(
                        "p (jp mb two m) -> p jp mb two m", jp=CK // 2, mb=MBLK, two=2
                    )
                    for jp in range(CK // 2):
                        for mb in range(MBLK):
                            mm_dr(at5, g, jp, mb)

    _split_excess_waits(nc)
    return nc


def _get_nc():
    if "nc" not in _cache:
        _cache["nc"] = _build()
    return _cache["nc"]


def make_in_maps(adj, features, W, b):
    import ml_dtypes

    adj = np.asarray(adj, dtype=np.float32)
    features = np.asarray(features, dtype=np.float32)
    W = np.asarray(W, dtype=np.float32)
    b = np.asarray(b, dtype=np.float32)

    featT = np.ascontiguousarray(features.T.astype(ml_dtypes.bfloat16))  # [fi, k]
    wt = np.ascontiguousarray(W.T.astype(ml_dtypes.bfloat16))  # [fi, fo]
    bias = np.ascontiguousarray(b.reshape(P, 1))

    in_maps = []
    half = GROUPS // 2
    for c in range(CORES):
        # [k, m] transpose of the row shard, centered; split into the even
        # (plain/e3m4) and odd (DoubleRow/e4m3) k-groups.
        arrT = np.ascontiguousarray(
            adj[c * ROWS : (c + 1) * ROWS, :].T
        ) - np.float32(0.5)
        bands = arrT.reshape(GROUPS, CK * P, ROWS)
        # plain: [gi, (j p), m] -> [gi, p, j, m]
        shardP = (
            bands[0::2]
            .astype(ml_dtypes.float8_e3m4)
            .reshape(half, CK, P, ROWS)
            .transpose(0, 2, 1, 3)
            .reshape(half * P, CK * ROWS)
        )
        # DoubleRow: [gi, (jp two p), m] -> [gi, p, jp, mb, two, m512]
        shardD = (
            bands[1::2]
            .astype(ml_dtypes.float8_e4m3)
            .reshape(half, CK // 2, 2, P, MBLK, 512)
            .transpose(0, 3, 1, 4, 2, 5)
            .reshape(half * P, CK * ROWS)
        )
        in_maps.append(
            {"adjP": shardP, "adjD": shardD, "featT": featT, "wt": wt, "bias": bias}
        )
    return in_maps


def assemble_output(results):
    out = np.empty((N, F), dtype=np.float32)
    for c in range(CORES):
        out[c * ROWS : (c + 1) * ROWS, :] = results[c]["outT"].astype(np.float32).T
    return out


def kernel(adj, features, W, b):
    from concourse.bass_utils import run_bass_kernel_spmd

    nc = _get_nc()
    in_maps = make_in_maps(adj, features, W, b)
    res = run_bass_kernel_spmd(nc, in_maps, list(range(CORES)))
    return assemble_output(res.results)


# revision 15
# speedup vs baseline: 1.1435x; 1.0219x over previous
"""DenseGCNConv on 8 Trainium2 NeuronCores (Bass/Tile).

out = (adj @ features) @ W.T + b,  adj [16384,16384] f32, features [16384,128],
W [128,128], b [128].

Strategy (row-parallel, per the sharding hint): core c owns rows
[c*2048, (c+1)*2048) of adj. Using associativity, out = adj @ fw + b with
fw = features @ W.T computed on-device (replicated on every core - it is
0.5 GFLOP vs 68 GFLOP total). The big operand adj is streamed from HBM
exactly once, as 1-byte fp8 of (adj - 0.5): centering the uniform-[0,1)
entries into [-0.5, 0.5) halves the quantization error. The exact
identity  adj @ fw = (adj - 0.5) @ fw + 0.5 * colsum(fw)  is restored
via a rank-1 correction folded into the bias:
  colsum(fw)[j] = sum_k fw[k,j] = (sum_k features[k,:]) @ W.T[:,j],
computed on device from a free-dim reduction of featT plus one 1-row
matmul.

With a 32 MiB/core fp8 stream the kernel is DMA-bound (~100us at ~370
GB/s/core) only if TensorE stays under that. Measured PE rates (this
hw): plain fp8 matmul = 0.43 ns/row contracting one 128-k chunk; fp8
DoubleRow = 0.43 ns/row contracting TWO chunks (2x work rate), but
DoubleRow needs BOTH operands e4m3 and fw as e4m3-only costs ~1.4e-2
extra error. So the 128 k-chunks are split 50/50, alternating per DMA
group:
  even groups: adj e3m4 x fw bf16, plain matmuls   (accurate, 1x rate)
  odd  groups: adj e4m3 x fw e4m3, DoubleRow pairs (2x rate)
=> PE ~85us < DMA ~100us, end-to-end rel err ~1.35e-2 (gate: 2e-2).
All 128 chunks accumulate into one PSUM region per m-block (mixed-mode
accumulation is just PSUM adds).

TensorE contracts over the partition dimension, so the streamed adj
tiles need K on partitions. The host packs each 1 MiB DMA group so each
partition's slice is contiguous: plain groups as [p, j, m]; DoubleRow
groups as [p, jp, mb, two, m512] so the moving AP [p, 2, 512] reads one
contiguous 1 KiB run per partition. Phase A (fw tiles from featT) is
emitted staggered ~4 groups ahead of first use so the in-order PE queue
never stalls on featT arrival or DVE casts. The per-core output
outT [128 fo, 2048 m] accumulates in 4 PSUM banks; one ACT pass adds
the corrected bias while copying PSUM->SBUF in bf16 (host upcasts).
"""

import sys

if "/opt/trn_rl_repo" not in sys.path:
    sys.path.insert(0, "/opt/trn_rl_repo")

import numpy as np

N = 16384
F = 128
P = 128
CORES = 8
ROWS = N // CORES  # 2048 rows of adj per core
KC = N // P  # 128 k-chunks
CK = 4  # k-chunks per DMA group (1 MiB per dma_start at fp8)
GROUPS = KC // CK  # 32
# Per-group mode: P = plain (adj e3m4 x fw bf16, 1x PE rate), D = DoubleRow
# (adj e4m3 x fw e4m3-hi, 2x PE rate). 12P/20D keeps rel err ~1.46e-2 (gate
# 2e-2) while cutting TensorE to ~87us so the PE never becomes the tail; the
# repeating pattern keeps PE pacing ahead of the DMA arrival rate.
PLAN = "PDDPDPDD" * 4
N_PLAIN = PLAN.count("P")  # 12
N_DR = PLAN.count("D")  # 20
MBLK = ROWS // 512  # 4 moving-operand blocks of 512
FEAT_G = N // 2048  # 8 featT DMA groups (also fw tiles, 16 k-chunks each)
ADJ_BUFS = 6  # buffering depth for the adj stream
FW_BUFS = 8  # fw ring depth (8 = fully resident)

_cache = {}


def configure(adj_bufs=None, fw_bufs=None):
    """Experiment knob: change buffering, invalidate caches."""
    global ADJ_BUFS, FW_BUFS
    if adj_bufs is not None:
        ADJ_BUFS = adj_bufs
    if fw_bufs is not None:
        FW_BUFS = fw_bufs
    _cache.clear()


def _split_excess_waits(nc, max_waits=1):
    """Walrus CoreV3 codegen rejects instructions with more than one SyncWait
    ("Too many sync wait commands"). Tile's kernel-tail drain accumulates one
    wait per semaphore lane; hoist the excess onto same-engine NoOps placed
    immediately before the offending instruction."""
    import concourse.mybir as mybir

    counter = [0]

    def fresh_name():
        counter[0] += 1
        return f"I-waitsplit-{counter[0]}"

    for fn in nc.m.functions:
        for blk in fn.blocks:
            new_insts = []
            for inst in blk.instructions:
                si = inst.sync_info
                if si is not None and si.on_wait and len(si.on_wait) > max_waits:
                    waits = list(si.on_wait)
                    extra, keep = waits[:-max_waits], waits[-max_waits:]
                    for i in range(0, len(extra), max_waits):
                        nop = mybir.InstNoOp(
                            name=fresh_name(),
                            engine=inst.engine,
                            sync_info=mybir.SyncInfo(
                                on_wait=extra[i : i + max_waits], on_update=[]
                            ),
                            bass_nofuse=True,
                        )
                        new_insts.append(nop)
                    si.on_wait = keep
                new_insts.append(inst)
            blk.instructions[:] = new_insts


def _build():
    import concourse.bass as bass
    import concourse.mybir as mybir
    from concourse.tile import TileContext

    f32 = mybir.dt.float32
    bf16 = mybir.dt.bfloat16
    e4 = mybir.dt.float8e4  # E4M3
    e3 = mybir.dt.float8e3  # E3M4: 4 mantissa bits
    DR = mybir.MatmulPerfMode.DoubleRow
    nc = bass.Bass()
    # Per-core adj shard, pre-transposed to [k, m] and packed per 1 MiB DMA
    # group; even k-groups (plain path) as e3m4 [g, p, j, m], odd k-groups
    # (DoubleRow path) as e4m3 [g, p, jp, mb, two, m512].
    adjP = nc.declare_dram_parameter(
        "adjP", [N_PLAIN * P, CK * ROWS], e3, isOutput=False
    )
    adjD = nc.declare_dram_parameter(
        "adjD", [N_DR * P, CK * ROWS], e4, isOutput=False
    )
    featT = nc.declare_dram_parameter("featT", [P, N], bf16, isOutput=False)
    wt = nc.declare_dram_parameter("wt", [P, F], bf16, isOutput=False)
    bias = nc.declare_dram_parameter("bias", [P, 1], f32, isOutput=False)
    outT = nc.declare_dram_parameter("outT", [P, ROWS], bf16, isOutput=True)

    with TileContext(nc) as tc:
        with (
            tc.tile_pool(name="const", bufs=1) as const_pool,
            tc.tile_pool(name="feat", bufs=2) as feat_pool,
            tc.tile_pool(name="fw", bufs=FW_BUFS) as fw_pool,
            tc.tile_pool(name="adj", bufs=ADJ_BUFS) as adj_pool,
            tc.tile_pool(name="outp", bufs=1) as out_pool,
            tc.tile_pool(name="psA", bufs=1, space="PSUM") as psA_pool,
            tc.tile_pool(name="psB", bufs=1, space="PSUM") as psB_pool,
        ):
            wt_sb = const_pool.tile([P, F], bf16)
            nc.scalar.dma_start(out=wt_sb, in_=wt[:])
            b_sb = const_pool.tile([P, 1], f32)
            nc.scalar.dma_start(out=b_sb, in_=bias[:])
            featsum8 = const_pool.tile([P, FEAT_G], f32)

            po = psB_pool.tile([P, ROWS], f32)
            o_sb = out_pool.tile([P, ROWS], bf16)
            adj_rP = adjP[:].rearrange("(G p) f -> G p f", p=P)
            adj_rD = adjD[:].rearrange("(G p) f -> G p f", p=P)

            fw_bf_tiles = []
            fw_hi_tiles = []

            def phase_a_tile(t):
                """fw[k, fo] for k-chunks 16t..16t+15: fp32 PSUM result cast
                to bf16 (plain path) and e4m3 (DoubleRow path). Alongside,
                the featT tile is free-dim-reduced into featsum8[:, t] for
                the centering correction."""
                ft = feat_pool.tile([P, 2048], bf16)
                eng = nc.sync if t % 2 == 0 else nc.scalar
                eng.dma_start(out=ft, in_=featT[:, t * 2048 : (t + 1) * 2048])
                pf = psA_pool.tile([P, 2048], f32, tag="pf")
                for j in range(2048 // F):
                    nc.tensor.matmul(
                        pf[:, j * F : (j + 1) * F],
                        lhsT=ft[:, j * F : (j + 1) * F],
                        rhs=wt_sb,
                        start=True,
                        stop=True,
                    )
                fw_bf = fw_pool.tile([P, 2048], bf16, tag="fwbf")
                nc.vector.tensor_copy(out=fw_bf, in_=pf)
                fw_hi = fw_pool.tile([P, 2048], e4, tag="fwhi")
                nc.vector.tensor_copy(out=fw_hi, in_=pf)
                nc.vector.tensor_reduce(
                    out=featsum8[:, t : t + 1],
                    in_=ft,
                    axis=mybir.AxisListType.X,
                    op=mybir.AluOpType.add,
                )
                fw_bf_tiles.append(fw_bf.rearrange("p (c f) -> p c f", f=F))
                fw_hi_tiles.append(fw_hi.rearrange("p (c f) -> p c f", f=F))

            def mm_plain(at, g, j, mb):
                ck = g * CK + j
                w = fw_bf_tiles[ck // 16]
                nc.tensor.matmul(
                    po[:, mb * 512 : (mb + 1) * 512],
                    lhsT=w[:, ck % 16, :],
                    rhs=at[:, j * ROWS + mb * 512 : j * ROWS + (mb + 1) * 512],
                    start=(ck == 0),
                    stop=False,
                )

            def mm_dr(at5, g, jp, mb):
                """DoubleRow: contracts k-chunks (ck, ck+1) for m-block mb."""
                ck = g * CK + 2 * jp
                w = fw_hi_tiles[ck // 16]
                c = ck % 16
                nc.tensor.matmul(
                    po[:, mb * 512 : (mb + 1) * 512],
                    lhsT=w[:, c : c + 2, :],
                    rhs=at5[:, jp, mb, :, :],
                    start=False,
                    stop=(ck == KC - 2),
                    perf_mode=DR,
                )

            phase_a_tile(0)
            phase_a_tile(1)
            for g in range(GROUPS):
                # Keep fw production one tile ahead of consumption.
                gpt = 16 // CK
                t = g // gpt
                if g % gpt == 0 and t >= 1 and t + 1 < FEAT_G:
                    phase_a_tile(t + 1)
                plain = PLAN[g] == "P"
                at = adj_pool.tile([P, CK * ROWS], e3 if plain else e4)
                dma_eng = nc.sync if g % 2 == 0 else nc.scalar
                src_idx = PLAN[:g].count("P") if plain else PLAN[:g].count("D")
                dma_eng.dma_start(
                    out=at, in_=(adj_rP if plain else adj_rD)[src_idx]
                )
                if g == GROUPS - 1:
                    # Centering correction: bias_eff = b + 0.5*(featsum@W.T).
                    featsum = const_pool.tile([P, 1], f32)
                    nc.vector.tensor_reduce(
                        out=featsum,
                        in_=featsum8,
                        axis=mybir.AxisListType.X,
                        op=mybir.AluOpType.add,
                    )
                    featsum_bf = const_pool.tile([P, 1], bf16)
                    nc.vector.tensor_copy(out=featsum_bf, in_=featsum)
                    # Same tag as pf so the bufs=1 ring reuses the slot
                    # instead of growing past the 8 PSUM banks; col 0 used.
                    psC_big = psA_pool.tile([P, 2048], f32, tag="pf")
                    psC = psC_big[:, 0:1]
                    nc.tensor.matmul(
                        psC, lhsT=wt_sb, rhs=featsum_bf, start=True, stop=True
                    )
                    bias_eff = const_pool.tile([P, 1], f32)
                    nc.scalar.activation(
                        bias_eff,
                        psC,
                        mybir.ActivationFunctionType.Identity,
                        bias=b_sb,
                        scale=0.5,
                    )
                    # Last group (odd => DoubleRow): finish one m-block at a
                    # time so the bias-add and output DMA of block mb overlap
                    # the matmuls of mb+1.
                    at5 = at.rearrange(
                        "p (jp mb two m) -> p jp mb two m", jp=CK // 2, mb=MBLK, two=2
                    )
                    for mb in range(MBLK):
                        for jp in range(CK // 2):
                            mm_dr(at5, g, jp, mb)
                        sl = slice(mb * 512, (mb + 1) * 512)
                        nc.scalar.activation(
                            o_sb[:, sl],
                            po[:, sl],
                            mybir.ActivationFunctionType.Identity,
                            bias=bias_eff,
                            scale=1.0,
                        )
                        nc.sync.dma_start(out=outT[:, sl], in_=o_sb[:, sl])
                elif plain:
                    for j in range(CK):
                        for mb in range(MBLK):
                            mm_plain(at, g, j, mb)
                else:
                    at5 = at.rearrange(
                        "p (jp mb two m) -> p jp mb two m", jp=CK // 2, mb=MBLK, two=2
                    )
                    for jp in range(CK // 2):
                        for mb in range(MBLK):
                            mm_dr(at5, g, jp, mb)

    _split_excess_waits(nc)
    return nc


def _get_nc():
    if "nc" not in _cache:
        _cache["nc"] = _build()
    return _cache["nc"]


def make_in_maps(adj, features, W, b):
    import ml_dtypes

    adj = np.asarray(adj, dtype=np.float32)
    features = np.asarray(features, dtype=np.float32)
    W = np.asarray(W, dtype=np.float32)
    b = np.asarray(b, dtype=np.float32)

    featT = np.ascontiguousarray(features.T.astype(ml_dtypes.bfloat16))  # [fi, k]
    wt = np.ascontiguousarray(W.T.astype(ml_dtypes.bfloat16))  # [fi, fo]
    bias = np.ascontiguousarray(b.reshape(P, 1))

    in_maps = []
    p_sel = [g for g in range(GROUPS) if PLAN[g] == "P"]
    d_sel = [g for g in range(GROUPS) if PLAN[g] == "D"]
    for c in range(CORES):
        # [k, m] transpose of the row shard, centered; split into the plain
        # (e3m4) and DoubleRow (e4m3) k-groups per PLAN.
        arrT = np.ascontiguousarray(
            adj[c * ROWS : (c + 1) * ROWS, :].T
        ) - np.float32(0.5)
        bands = arrT.reshape(GROUPS, CK * P, ROWS)
        # plain: [gi, (j p), m] -> [gi, p, j, m]
        shardP = (
            bands[p_sel]
            .astype(ml_dtypes.float8_e3m4)
            .reshape(N_PLAIN, CK, P, ROWS)
            .transpose(0, 2, 1, 3)
            .reshape(N_PLAIN * P, CK * ROWS)
        )
        # DoubleRow: [gi, (jp two p), m] -> [gi, p, jp, mb, two, m512]
        shardD = (
            bands[d_sel]
            .astype(ml_dtypes.float8_e4m3)
            .reshape(N_DR, CK // 2, 2, P, MBLK, 512)
            .transpose(0, 3, 1, 4, 2, 5)
            .reshape(N_DR * P, CK * ROWS)
        )
        in_maps.append(
            {"adjP": shardP, "adjD": shardD, "featT": featT, "wt": wt, "bias": bias}
        )
    return in_maps


def assemble_output(results):
    out = np.empty((N, F), dtype=np.float32)
    for c in range(CORES):
        out[c * ROWS : (c + 1) * ROWS, :] = results[c]["outT"].astype(np.float32).T
    return out


def kernel(adj, features, W, b):
    from concourse.bass_utils import run_bass_kernel_spmd

    nc = _get_nc()
    in_maps = make_in_maps(adj, features, W, b)
    res = run_bass_kernel_spmd(nc, in_maps, list(range(CORES)))
    return assemble_output(res.results)


# revision 16
# speedup vs baseline: 1.1520x; 1.0074x over previous
"""DenseGCNConv on 8 Trainium2 NeuronCores (Bass/Tile).

out = (adj @ features) @ W.T + b,  adj [16384,16384] f32, features [16384,128],
W [128,128], b [128].

Strategy (row-parallel, per the sharding hint): core c owns rows
[c*2048, (c+1)*2048) of adj. Using associativity, out = adj @ fw + b with
fw = features @ W.T computed on-device (replicated on every core - it is
0.5 GFLOP vs 68 GFLOP total). The big operand adj is streamed from HBM
exactly once, as 1-byte fp8 of (adj - 0.5): centering the uniform-[0,1)
entries into [-0.5, 0.5) halves the quantization error. The exact
identity  adj @ fw = (adj - 0.5) @ fw + 0.5 * colsum(fw)  is restored
via a rank-1 correction folded into the bias:
  colsum(fw)[j] = sum_k fw[k,j] = (sum_k features[k,:]) @ W.T[:,j],
computed on device from a free-dim reduction of featT plus one 1-row
matmul.

With a 32 MiB/core fp8 stream the kernel is DMA-bound (~100us at ~370
GB/s/core) only if TensorE stays under that. Measured PE rates (this
hw): plain fp8 matmul = 0.43 ns/row contracting one 128-k chunk; fp8
DoubleRow = 0.43 ns/row contracting TWO chunks (2x work rate), but
DoubleRow needs BOTH operands e4m3 and fw as e4m3-only costs ~1.4e-2
extra error. So the 128 k-chunks are split 50/50, alternating per DMA
group:
  even groups: adj e3m4 x fw bf16, plain matmuls   (accurate, 1x rate)
  odd  groups: adj e4m3 x fw e4m3, DoubleRow pairs (2x rate)
=> PE ~85us < DMA ~100us, end-to-end rel err ~1.35e-2 (gate: 2e-2).
All 128 chunks accumulate into one PSUM region per m-block (mixed-mode
accumulation is just PSUM adds).

TensorE contracts over the partition dimension, so the streamed adj
tiles need K on partitions. The host packs each 1 MiB DMA group so each
partition's slice is contiguous: plain groups as [p, j, m]; DoubleRow
groups as [p, jp, mb, two, m512] so the moving AP [p, 2, 512] reads one
contiguous 1 KiB run per partition. Phase A (fw tiles from featT) is
emitted staggered ~4 groups ahead of first use so the in-order PE queue
never stalls on featT arrival or DVE casts. The per-core output
outT [128 fo, 2048 m] accumulates in 4 PSUM banks; one ACT pass adds
the corrected bias while copying PSUM->SBUF in bf16 (host upcasts).
"""

import sys

if "/opt/trn_rl_repo" not in sys.path:
    sys.path.insert(0, "/opt/trn_rl_repo")

import numpy as np

N = 16384
F = 128
P = 128
CORES = 8
ROWS = N // CORES  # 2048 rows of adj per core
KC = N // P  # 128 k-chunks
CK = 4  # k-chunks per DMA group (1 MiB per dma_start at fp8)
GROUPS = KC // CK  # 32
# Per-group mode: P = plain (adj e3m4 x fw bf16, 1x PE rate), D = DoubleRow
# (adj e4m3 x fw e4m3-hi, 2x PE rate). 8P/24D keeps rel err ~1.56e-2 (gate
# 2e-2) while cutting TensorE to ~80us so the PE never becomes the tail; the
# repeating pattern keeps PE pacing ahead of the DMA arrival rate.
PLAN = "PDDD" * 8
N_PLAIN = PLAN.count("P")  # 8
N_DR = PLAN.count("D")  # 24
MBLK = ROWS // 512  # 4 moving-operand blocks of 512
FEAT_G = N // 2048  # 8 featT DMA groups (also fw tiles, 16 k-chunks each)
ADJ_BUFS = 10  # buffering depth for the adj stream
FW_BUFS = 8  # fw ring depth (8 = fully resident)

_cache = {}


def configure(adj_bufs=None, fw_bufs=None):
    """Experiment knob: change buffering, invalidate caches."""
    global ADJ_BUFS, FW_BUFS
    if adj_bufs is not None:
        ADJ_BUFS = adj_bufs
    if fw_bufs is not None:
        FW_BUFS = fw_bufs
    _cache.clear()


def _split_excess_waits(nc, max_waits=1):
    """Walrus CoreV3 codegen rejects instructions with more than one SyncWait
    ("Too many sync wait commands"). Tile's kernel-tail drain accumulates one
    wait per semaphore lane; hoist the excess onto same-engine NoOps placed
    immediately before the offending instruction."""
    import concourse.mybir as mybir

    counter = [0]

    def fresh_name():
        counter[0] += 1
        return f"I-waitsplit-{counter[0]}"

    for fn in nc.m.functions:
        for blk in fn.blocks:
            new_insts = []
            for inst in blk.instructions:
                si = inst.sync_info
                if si is not None and si.on_wait and len(si.on_wait) > max_waits:
                    waits = list(si.on_wait)
                    extra, keep = waits[:-max_waits], waits[-max_waits:]
                    for i in range(0, len(extra), max_waits):
                        nop = mybir.InstNoOp(
                            name=fresh_name(),
                            engine=inst.engine,
                            sync_info=mybir.SyncInfo(
                                on_wait=extra[i : i + max_waits], on_update=[]
                            ),
                            bass_nofuse=True,
                        )
                        new_insts.append(nop)
                    si.on_wait = keep
                new_insts.append(inst)
            blk.instructions[:] = new_insts


def _build():
    import concourse.bass as bass
    import concourse.mybir as mybir
    from concourse.tile import TileContext

    f32 = mybir.dt.float32
    bf16 = mybir.dt.bfloat16
    e4 = mybir.dt.float8e4  # E4M3
    e3 = mybir.dt.float8e3  # E3M4: 4 mantissa bits
    DR = mybir.MatmulPerfMode.DoubleRow
    nc = bass.Bass()
    # Per-core adj shard, pre-transposed to [k, m] and packed per 1 MiB DMA
    # group; even k-groups (plain path) as e3m4 [g, p, j, m], odd k-groups
    # (DoubleRow path) as e4m3 [g, p, jp, mb, two, m512].
    adjP = nc.declare_dram_parameter(
        "adjP", [N_PLAIN * P, CK * ROWS], e3, isOutput=False
    )
    adjD = nc.declare_dram_parameter(
        "adjD", [N_DR * P, CK * ROWS], e4, isOutput=False
    )
    featT = nc.declare_dram_parameter("featT", [P, N], bf16, isOutput=False)
    wt = nc.declare_dram_parameter("wt", [P, F], bf16, isOutput=False)
    bias = nc.declare_dram_parameter("bias", [P, 1], f32, isOutput=False)
    outT = nc.declare_dram_parameter("outT", [P, ROWS], bf16, isOutput=True)

    with TileContext(nc) as tc:
        with (
            tc.tile_pool(name="const", bufs=1) as const_pool,
            tc.tile_pool(name="feat", bufs=2) as feat_pool,
            tc.tile_pool(name="fw", bufs=FW_BUFS) as fw_pool,
            tc.tile_pool(name="adj", bufs=ADJ_BUFS) as adj_pool,
            tc.tile_pool(name="outp", bufs=1) as out_pool,
            tc.tile_pool(name="psA", bufs=1, space="PSUM") as psA_pool,
            tc.tile_pool(name="psB", bufs=1, space="PSUM") as psB_pool,
        ):
            wt_sb = const_pool.tile([P, F], bf16)
            nc.scalar.dma_start(out=wt_sb, in_=wt[:])
            b_sb = const_pool.tile([P, 1], f32)
            nc.scalar.dma_start(out=b_sb, in_=bias[:])
            featsum8 = const_pool.tile([P, FEAT_G], f32)

            po = psB_pool.tile([P, ROWS], f32)
            o_sb = out_pool.tile([P, ROWS], bf16)
            adj_rP = adjP[:].rearrange("(G p) f -> G p f", p=P)
            adj_rD = adjD[:].rearrange("(G p) f -> G p f", p=P)

            fw_bf_tiles = []
            fw_hi_tiles = []

            def phase_a_tile(t):
                """fw[k, fo] for k-chunks 16t..16t+15: fp32 PSUM result cast
                to bf16 (plain path) and e4m3 (DoubleRow path). Alongside,
                the featT tile is free-dim-reduced into featsum8[:, t] for
                the centering correction."""
                ft = feat_pool.tile([P, 2048], bf16)
                eng = nc.sync if t % 2 == 0 else nc.scalar
                eng.dma_start(out=ft, in_=featT[:, t * 2048 : (t + 1) * 2048])
                pf = psA_pool.tile([P, 2048], f32, tag="pf")
                for j in range(2048 // F):
                    nc.tensor.matmul(
                        pf[:, j * F : (j + 1) * F],
                        lhsT=ft[:, j * F : (j + 1) * F],
                        rhs=wt_sb,
                        start=True,
                        stop=True,
                    )
                fw_bf = fw_pool.tile([P, 2048], bf16, tag="fwbf")
                nc.vector.tensor_copy(out=fw_bf, in_=pf)
                fw_hi = fw_pool.tile([P, 2048], e4, tag="fwhi")
                nc.vector.tensor_copy(out=fw_hi, in_=pf)
                nc.vector.tensor_reduce(
                    out=featsum8[:, t : t + 1],
                    in_=ft,
                    axis=mybir.AxisListType.X,
                    op=mybir.AluOpType.add,
                )
                fw_bf_tiles.append(fw_bf.rearrange("p (c f) -> p c f", f=F))
                fw_hi_tiles.append(fw_hi.rearrange("p (c f) -> p c f", f=F))

            def mm_plain(at, g, j, mb):
                ck = g * CK + j
                w = fw_bf_tiles[ck // 16]
                nc.tensor.matmul(
                    po[:, mb * 512 : (mb + 1) * 512],
                    lhsT=w[:, ck % 16, :],
                    rhs=at[:, j * ROWS + mb * 512 : j * ROWS + (mb + 1) * 512],
                    start=(ck == 0),
                    stop=False,
                )

            def mm_dr(at5, g, jp, mb):
                """DoubleRow: contracts k-chunks (ck, ck+1) for m-block mb."""
                ck = g * CK + 2 * jp
                w = fw_hi_tiles[ck // 16]
                c = ck % 16
                nc.tensor.matmul(
                    po[:, mb * 512 : (mb + 1) * 512],
                    lhsT=w[:, c : c + 2, :],
                    rhs=at5[:, jp, mb, :, :],
                    start=False,
                    stop=(ck == KC - 2),
                    perf_mode=DR,
                )

            phase_a_tile(0)
            phase_a_tile(1)
            for g in range(GROUPS):
                # Keep fw production one tile ahead of consumption.
                gpt = 16 // CK
                t = g // gpt
                if g % gpt == 0 and t >= 1 and t + 1 < FEAT_G:
                    phase_a_tile(t + 1)
                plain = PLAN[g] == "P"
                at = adj_pool.tile([P, CK * ROWS], e3 if plain else e4)
                dma_eng = nc.sync if g % 2 == 0 else nc.scalar
                src_idx = PLAN[:g].count("P") if plain else PLAN[:g].count("D")
                dma_eng.dma_start(
                    out=at, in_=(adj_rP if plain else adj_rD)[src_idx]
                )
                if g == GROUPS - 1:
                    # Centering correction: bias_eff = b + 0.5*(featsum@W.T).
                    featsum = const_pool.tile([P, 1], f32)
                    nc.vector.tensor_reduce(
                        out=featsum,
                        in_=featsum8,
                        axis=mybir.AxisListType.X,
                        op=mybir.AluOpType.add,
                    )
                    featsum_bf = const_pool.tile([P, 1], bf16)
                    nc.vector.tensor_copy(out=featsum_bf, in_=featsum)
                    # Same tag as pf so the bufs=1 ring reuses the slot
                    # instead of growing past the 8 PSUM banks; col 0 used.
                    psC_big = psA_pool.tile([P, 2048], f32, tag="pf")
                    psC = psC_big[:, 0:1]
                    nc.tensor.matmul(
                        psC, lhsT=wt_sb, rhs=featsum_bf, start=True, stop=True
                    )
                    bias_eff = const_pool.tile([P, 1], f32)
                    nc.scalar.activation(
                        bias_eff,
                        psC,
                        mybir.ActivationFunctionType.Identity,
                        bias=b_sb,
                        scale=0.5,
                    )
                    # Last group (odd => DoubleRow): finish one m-block at a
                    # time so the bias-add and output DMA of block mb overlap
                    # the matmuls of mb+1.
                    at5 = at.rearrange(
                        "p (jp mb two m) -> p jp mb two m", jp=CK // 2, mb=MBLK, two=2
                    )
                    for mb in range(MBLK):
                        for jp in range(CK // 2):
                            mm_dr(at5, g, jp, mb)
                        sl = slice(mb * 512, (mb + 1) * 512)
                        nc.scalar.activation(
                            o_sb[:, sl],
                            po[:, sl],
                            mybir.ActivationFunctionType.Identity,
                            bias=bias_eff,
                            scale=1.0,
                        )
                        nc.sync.dma_start(out=outT[:, sl], in_=o_sb[:, sl])
                elif plain:
                    for j in range(CK):
                        for mb in range(MBLK):
                            mm_plain(at, g, j, mb)
                else:
                    at5 = at.rearrange(
                        "p (jp mb two m) -> p jp mb two m", jp=CK // 2, mb=MBLK, two=2
                    )
                    for jp in range(CK // 2):
                        for mb in range(MBLK):
                            mm_dr(at5, g, jp, mb)

    _split_excess_waits(nc)
    return nc


def _get_nc():
    if "nc" not in _cache:
        _cache["nc"] = _build()
    return _cache["nc"]


def make_in_maps(adj, features, W, b):
    import ml_dtypes

    adj = np.asarray(adj, dtype=np.float32)
    features = np.asarray(features, dtype=np.float32)
    W = np.asarray(W, dtype=np.float32)
    b = np.asarray(b, dtype=np.float32)

    featT = np.ascontiguousarray(features.T.astype(ml_dtypes.bfloat16))  # [fi, k]
    wt = np.ascontiguousarray(W.T.astype(ml_dtypes.bfloat16))  # [fi, fo]
    bias = np.ascontiguousarray(b.reshape(P, 1))

    in_maps = []
    p_sel = [g for g in range(GROUPS) if PLAN[g] == "P"]
    d_sel = [g for g in range(GROUPS) if PLAN[g] == "D"]
    for c in range(CORES):
        # [k, m] transpose of the row shard, centered; split into the plain
        # (e3m4) and DoubleRow (e4m3) k-groups per PLAN.
        arrT = np.ascontiguousarray(
            adj[c * ROWS : (c + 1) * ROWS, :].T
        ) - np.float32(0.5)
        bands = arrT.reshape(GROUPS, CK * P, ROWS)
        # plain: [gi, (j p), m] -> [gi, p, j, m]
        shardP = (
            bands[p_sel]
            .astype(ml_dtypes.float8_e3m4)
            .reshape(N_PLAIN, CK, P, ROWS)
            .transpose(0, 2, 1, 3)
            .reshape(N_PLAIN * P, CK * ROWS)
        )
        # DoubleRow: [gi, (jp two p), m] -> [gi, p, jp, mb, two, m512]
        shardD = (
            bands[d_sel]
            .astype(ml_dtypes.float8_e4m3)
            .reshape(N_DR, CK // 2, 2, P, MBLK, 512)
            .transpose(0, 3, 1, 4, 2, 5)
            .reshape(N_DR * P, CK * ROWS)
        )
        in_maps.append(
            {"adjP": shardP, "adjD": shardD, "featT": featT, "wt": wt, "bias": bias}
        )
    return in_maps


def assemble_output(results):
    out = np.empty((N, F), dtype=np.float32)
    for c in range(CORES):
        out[c * ROWS : (c + 1) * ROWS, :] = results[c]["outT"].astype(np.float32).T
    return out


def kernel(adj, features, W, b):
    from concourse.bass_utils import run_bass_kernel_spmd

    nc = _get_nc()
    in_maps = make_in_maps(adj, features, W, b)
    res = run_bass_kernel_spmd(nc, in_maps, list(range(CORES)))
    return assemble_output(res.results)
